# revision 1
# baseline (speedup 1.0000x reference)
"""Trainium2 Bass kernel for nn_CausePredictor (RGCN + pairwise MLP).

Sharding: data-parallel over the pairwise row index i (dim 1 of the
[B,S,S] output): 8 cores x 25 rows, replicated over B=4.  All per-core
differences are encoded as input DATA (column slices / gathered pe
tables), so one SPMD program serves all cores.

Math (matching reference.py):
  h   = sum_k Ahat_k.T @ (x[b] @ basis_k) + x[b] @ root + bias
  u   = h @ W1a   (j term),  v = h @ W1c  (i term)
  T   = pe_k @ W1b + pe_v @ W1d          # [11, 512], host precomputed
  h1[b,i,j,:] = u[b,j] + v[b,i] + T[pos(i,j)]
  out = sigmoid(Wp . relu(relu(h1) @ W2)) * mask

Performance structure (per TimelineSim cost model):
  - stage A (RGCN + u/v) runs in bf16 (fp32 matmuls are 4x slower).
  - peR = T[pos] pair-expansion is precomputed on HOST and DMA'd in.
  - t = u + peR is ONE broadcast tensor_add per (b, mc) on DVE.
  - rh1 = relu(t + v_i): K-chunks 0,1 quantize to fp8 (on Pool),
    chunks 2,3 stay bf16 (on DVE).
  - GEMM2: fp8 chunks use DoubleRow matmuls (0.5 cyc/row) with W2
    split into hi+lo fp8 pair (weight quantization ~exact); bf16
    chunks use normal matmuls.  3 cyc/row total vs 4 for pure bf16.
  - relu2 PSUM->SBUF copies split 3:1 between Activation and Pool.
  - GEMM3 (the Wp dot) runs with h2 chunks STATIONARY and wp moving:
    out is [pairs<=100, 1], ~1 PE row per matmul instead of 400.
"""

import sys

sys.path.insert(0, "/opt/trn_rl_repo")

import numpy as np

B, S, D, M, P = 4, 200, 300, 512, 100
NREL, MAXL = 9, 10
NCORES = 8
IPC = S // NCORES  # 25 rows of i per core
NU = IPC // 2 + 1  # 13 units per b: 12x 2-row + 1x 1-row
FPC = IPC * S  # 5000 pairs per (b, core)
NCOL = 50  # output columns per b: each col = 100 pairs

_prog_cache = {}


def _rel_adj(s):
    ra = np.arange(s)[None, :] - np.arange(s)[:, None]
    for i in range(s):
        ra[i, i + 1 :] = 1
        num = 1
        for o in range(i - 1, -1, -2):
            ra[i, o] = -num
            if o - 1 >= 0:
                ra[i, o - 1] = -num
            num += 1
        ra[i, :i] = np.maximum(ra[i, :i], -8)  # -(WINDOW+1), WINDOW=7
    return ra


def _pack_k(w, width=None):
    """[K, N] -> [128, ceil(K/128)*N], K chunked onto partitions, zero pad."""
    k, n = w.shape
    nch = (k + 127) // 128
    out = np.zeros((128, nch * n), np.float32)
    for c in range(nch):
        r = min(128, k - c * 128)
        out[:r, c * n : c * n + n] = w[c * 128 : c * 128 + r]
    return out


def _build_program():
    import ml_dtypes  # noqa: F401
    import concourse.tile as tile
    from concourse import bacc, mybir

    f32 = mybir.dt.float32
    bf16 = mybir.dt.bfloat16
    fp8 = mybir.dt.float8e4
    AF = mybir.ActivationFunctionType
    OP = mybir.AluOpType
    PM = mybir.MatmulPerfMode

    nc = bacc.Bacc()

    SC = S + IPC  # 225: concat of all-j columns and the core's i-slice
    dxT = nc.declare_dram_parameter("xT", [D, B * SC], bf16, isOutput=False)
    dahat = nc.declare_dram_parameter("ahat", [128, 4 * SC], bf16, isOutput=False)
    dbasis = nc.declare_dram_parameter("basis", [128, 1800], bf16, isOutput=False)
    droot = nc.declare_dram_parameter("root", [128, 900], bf16, isOutput=False)
    dbias = nc.declare_dram_parameter("bias", [128, 3], f32, isOutput=False)
    dw1a = nc.declare_dram_parameter("w1a", [128, 1536], bf16, isOutput=False)
    dw1c = nc.declare_dram_parameter("w1c", [128, 1536], bf16, isOutput=False)
    dw2f8 = nc.declare_dram_parameter("w2f8", [128, 2048], fp8, isOutput=False)
    dw2b = nc.declare_dram_parameter("w2b", [128, 2 * M], bf16, isOutput=False)
    dwp = nc.declare_dram_parameter("wp", [128, 4], bf16, isOutput=False)
    dper = nc.declare_dram_parameter("per", [128, 4 * FPC], bf16, isOutput=False)
    dout = nc.declare_dram_parameter("out", [B * 100, NCOL], f32, isOutput=True)

    DCW = [128, 128, 44]  # D=300 chunks
    JCW = [128, 72]  # S=200 chunks

    with tile.TileContext(nc) as tc:
        with (
            tc.tile_pool(name="persist", bufs=1) as pp,
            tc.tile_pool(name="tpool", bufs=2) as tp,
            tc.tile_pool(name="work", bufs=3) as pwork,
            tc.tile_pool(name="sigp", bufs=2) as psig,
        ):
            def load(name, shape, dt, src):
                t = pp.tile(shape, dt, tag=name, name=name)
                if len(shape) == 3:
                    nc.sync.dma_start(t[:, :, :], src)
                else:
                    nc.sync.dma_start(t[:, :], src)
                return t

            # DMA order = deadline order.  The serial DMA stream (~23us for
            # 8MB) is a startup critical path: stage-A weights first, then
            # the first-half rows of peR (consumed by the early units),
            # then stage-B weights, then peR's second halves (not needed
            # until ~unit 7 of b=0).
            CH = 13 * S  # peR column split matching the bulk-TT halves
            basis = load("basis", [128, 1800], bf16, dbasis[:, :])
            xT = [load(f"xT{c}", [DCW[c], B * SC], bf16,
                       dxT[c * 128 : c * 128 + DCW[c], :]) for c in range(3)]
            ahat = load("ahat", [128, 4 * SC], bf16, dahat[:, :])
            root = load("root", [128, 900], bf16, droot[:, :])
            bias = load("bias", [128, 3], f32, dbias[:, :])
            w1a = load("w1a", [128, 1536], bf16, dw1a[:, :])
            w1c = load("w1c", [128, 1536], bf16, dw1c[:, :])
            peR = [pp.tile([128, FPC], bf16, tag=f"peR{mc}", name=f"peR{mc}")
                   for mc in range(4)]
            for mc in range(4):
                nc.sync.dma_start(peR[mc][:, 0:CH], dper[:, mc * FPC : mc * FPC + CH])
            # fp8 GEMM2 weights: one tile, sliced [128, 2, 128] per (s, n)
            w28t = load("w28t", [128, 2048], fp8, dw2f8[:, :])
            w28 = [[w28t[:, (s * 4 + n) * 256 : (s * 4 + n) * 256 + 256]
                    .rearrange("p (two m) -> p two m", two=2)
                    for n in range(4)] for s in range(2)]
            w2b = load("w2b", [128, 2 * M], bf16, dw2b[:, :])
            for mc in range(4):
                nc.sync.dma_start(peR[mc][:, CH:FPC],
                                  dper[:, mc * FPC + CH : (mc + 1) * FPC])
            wp = load("wp", [128, 4], bf16, dwp[:, :])

            hT = [[pp.tile([DCW[ec], SC], bf16, tag=f"hT{b}{ec}", name=f"hT{b}{ec}")
                   for ec in range(3)] for b in range(B)]
            # u (cols 0:S) and v (cols S:SC) in ONE tile per (b, mc) so the
            # PSUM->SBUF copy is a single instruction.
            uvT = [[pp.tile([128, SC], bf16, tag=f"uvT{b}{mc}", name=f"uvT{b}{mc}")
                    for mc in range(4)] for b in range(B)]
            vT = [[pp.tile([128, IPC], f32, tag=f"vT{b}{mc}", name=f"vT{b}{mc}")
                   for mc in range(4)] for b in range(B)]

            # bulk-TT parts: t[b][mc] = u[b][mc] (broadcast over i) + peR[mc],
            # emitted in 8 half-row pieces so DVE is never blocked for long.
            tbs = {}
            HROWS = [(0, 13), (13, IPC)]

            def emit_tt_part(b, part):
                if b not in tbs:
                    tbs[b] = tp.tile([128, 4, FPC], bf16, tag="tb", name=f"tb{b}")
                tb = tbs[b]
                half, mc = divmod(part, 4)
                r0, r1 = HROWS[half]
                rows = r1 - r0
                nc.vector.tensor_add(
                    tb[:, mc, r0 * S : r1 * S].rearrange("p (r j) -> p r j", r=rows),
                    uvT[b][mc][:, 0:S].unsqueeze(1).broadcast_to([128, rows, S]),
                    peR[mc][:, r0 * S : r1 * S].rearrange("p (r j) -> p r j", r=rows))

            # ---------------- stage A: RGCN h, then u/v ----------
            with tc.tile_pool(name="psA", bufs=2, space="PSUM") as psA:
                t1 = [[[pp.tile([JCW[jc], D], bf16, tag=f"t1_{b}{k}{jc}",
                                name=f"t1_{b}{k}{jc}")
                        for jc in range(2)] for k in range(2)] for b in range(B)]

                def emit_t1(b):
                    for k in range(2):
                        for jc in range(2):
                            t1ps = psA.tile([JCW[jc], D], f32, tag="mps", name="t1ps")
                            for dc in range(3):
                                nc.tensor.matmul(
                                    t1ps[:, :],
                                    xT[dc][:, b * SC + jc * 128 : b * SC + jc * 128 + JCW[jc]],
                                    basis[0 : DCW[dc], (k * 3 + dc) * D : (k * 3 + dc) * D + D],
                                    start=(dc == 0), stop=(dc == 2),
                                )
                            if b == 0:
                                nc.vector.tensor_copy(t1[b][k][jc][:, :], t1ps[:, :])
                            else:
                                nc.scalar.activation(t1[b][k][jc][:, :], t1ps[:, :], AF.Copy)

                def emit_h(b):
                    for ec in range(3):
                        hps = psA.tile([DCW[ec], SC], f32, tag="hps", name="hps")
                        first = True
                        for k in range(2):
                            for jc in range(2):
                                nc.tensor.matmul(
                                    hps[:, :],
                                    t1[b][k][jc][:, ec * 128 : ec * 128 + DCW[ec]],
                                    ahat[0 : JCW[jc], (k * 2 + jc) * SC : (k * 2 + jc + 1) * SC],
                                    start=first, stop=False)
                                first = False
                        for dc in range(3):
                            nc.tensor.matmul(
                                hps[:, :],
                                root[0 : DCW[dc], dc * D + ec * 128 : dc * D + ec * 128 + DCW[ec]],
                                xT[dc][:, b * SC : (b + 1) * SC],
                                start=False, stop=(dc == 2))
                        if b == 0:
                            nc.vector.tensor_scalar(
                                out=hT[b][ec][:, :], in0=hps[:, :],
                                scalar1=bias[0 : DCW[ec], ec : ec + 1],
                                scalar2=None, op0=OP.add)
                        else:
                            nc.scalar.activation(hT[b][ec][:, :], hps[:, :], AF.Identity,
                                                 bias=bias[0 : DCW[ec], ec : ec + 1])

                def emit_uv(b):
                    # b=0's copies stay on Act (its critical path); later
                    # batches copy on DVE so Act's copy chain never gates
                    # the PE through the stage-A tail.
                    for mc in range(4):
                        ups = psA.tile([128, SC], f32, tag="uvps", name="ups")
                        for ec in range(3):
                            nc.tensor.matmul(
                                ups[:, 0:S],
                                w1a[0 : DCW[ec], ec * M + mc * 128 : ec * M + mc * 128 + 128],
                                hT[b][ec][:, 0:S], start=(ec == 0), stop=False)
                        for ec in range(3):
                            nc.tensor.matmul(
                                ups[:, S:SC],
                                w1c[0 : DCW[ec], ec * M + mc * 128 : ec * M + mc * 128 + 128],
                                hT[b][ec][:, S:SC], start=(ec == 0), stop=(ec == 2))
                        if b == 0:
                            nc.vector.tensor_copy(uvT[b][mc][:, :], ups[:, :])
                            nc.vector.tensor_copy(vT[b][mc][:, :], ups[:, S:SC])
                        else:
                            nc.scalar.activation(uvT[b][mc][:, :], ups[:, :], AF.Copy)
                            nc.scalar.activation(vT[b][mc][:, :], ups[:, S:SC], AF.Copy)

                # b=0 chain first so its bulk-TT (DVE) overlaps the rest of
                # stage A on the PE.
                emit_t1(0)
                emit_h(0)
                emit_uv(0)
                for part in range(4):  # h0 rows; h1 parts go inside stage B
                    emit_tt_part(0, part)
                for b in range(1, B):
                    emit_t1(b)
                for b in range(1, B):
                    emit_h(b)
                for b in range(1, B):
                    emit_uv(b)

            # ---------------- stage B: the pairwise MLP ------------------
            with (
                tc.tile_pool(name="ps2", bufs=5, space="PSUM") as ps2,
                tc.tile_pool(name="pp3", bufs=2, space="PSUM") as pp3,
            ):
                def emit_g3(prev):
                    # GEMM3 for the previous unit: h2 chunks stationary,
                    # wp moving -> out [pairs<=100, 1] into the b's pout col.
                    p_u, p_nch, p_pout, p_rh2 = prev
                    for pc in range(p_nch):
                        col = p_u * 4 + pc
                        for mc in range(4):
                            nc.tensor.matmul(
                                p_pout[0:100, col : col + 1],
                                p_rh2[mc][:, pc * 100 : pc * 100 + 100],
                                wp[:, mc : mc + 1],
                                start=(mc == 0), stop=(mc == 3))

                prev = None
                for b in range(B):
                    tb = tbs[b]
                    pout = pp3.tile([128, NCOL], f32, tag="pout", name="pout")
                    sig = psig.tile([128, NCOL], f32, tag="sigb", name="sigb")
                    for u in range(NU):
                        nil = 2 if u < NU - 1 else 1
                        ncols = nil * S
                        # rh1: fp8 for K-chunks 0,1 (Pool), bf16 for 2,3 (DVE)
                        r8 = pwork.tile([128, 2, 400], fp8, tag="r8", name="r8")
                        rb = [pwork.tile([128, 400], bf16, tag=f"rb{j}", name=f"rb{j}")
                              for j in range(2)]
                        for mc in range(4):
                            for h in range(nil):
                                src = tb[:, mc, u * 400 + h * S : u * 400 + h * S + S]
                                vsc = vT[b][mc][:, 2 * u + h : 2 * u + h + 1]
                                if mc < 2:
                                    nc.gpsimd.tensor_scalar(
                                        out=r8[:, mc, h * S : h * S + S],
                                        in0=src, scalar1=vsc,
                                        scalar2=0.0, op0=OP.add, op1=OP.max)
                                else:
                                    nc.vector.tensor_scalar(
                                        out=rb[mc - 2][:, h * S : h * S + S],
                                        in0=src, scalar1=vsc,
                                        scalar2=0.0, op0=OP.add, op1=OP.max)
                        # GEMM2 + relu2
                        rh2 = [pwork.tile([128, 400], bf16, tag=f"rh2_{n}", name=f"rh2_{n}")
                               for n in range(4)]
                        for n in range(4):
                            ops = ps2.tile([128, 400], f32, tag="ops", name="ops")
                            for s in range(2):
                                nc.tensor.matmul(
                                    ops[:, :ncols],
                                    w28[s][n][:, :, :],
                                    r8[:, :, :ncols],
                                    start=(s == 0), stop=False,
                                    perf_mode=PM.DoubleRow)
                            for j in range(2):
                                nc.tensor.matmul(
                                    ops[:, :ncols],
                                    w2b[:, j * M + n * 128 : j * M + n * 128 + 128],
                                    rb[j][:, :ncols],
                                    start=False, stop=(j == 1))
                            # relu2 split: DVE has slack in later batches
                            # (no more bulk-TT), Act is the constraint there.
                            on_dve = ((b < 2 and n == 3 and u % 2 == 0)
                                      or (b == 2 and n == 3)
                                      or (b == 3 and n >= 2))
                            if on_dve:
                                nc.vector.tensor_scalar(
                                    out=rh2[n][:, :ncols], in0=ops[:, :ncols],
                                    scalar1=0.0, scalar2=None, op0=OP.max)
                            else:
                                nc.scalar.activation(rh2[n][:, :ncols], ops[:, :ncols], AF.Relu)
                        # GEMM3 of the PREVIOUS unit (hides relu2 latency)
                        if prev is not None:
                            emit_g3(prev)
                        prev = (u, 4 if nil == 2 else 2, pout, rh2)
                        # b=0's TT second-half parts wait for the late peR
                        # columns; emit them where the DMA has landed.
                        if b == 0 and 2 <= u <= 5:
                            emit_tt_part(0, 4 + (u - 2))
                        # spread the next batch's bulk-TT parts over units 2..9
                        if 2 <= u <= 9 and b + 1 < B:
                            emit_tt_part(b + 1, u - 2)
                        # first 6 units' output cols are final once g3(5) ran
                        if u == 7:
                            nc.scalar.activation(sig[0:100, 0:24], pout[0:100, 0:24],
                                                 AF.Sigmoid)
                            nc.sync.dma_start(dout[b * 100 : b * 100 + 100, 0:24],
                                              sig[0:100, 0:24])
                    # cols 24:48 are final after g3(11) (emitted in the u=12
                    # iteration above); only unit 12's 2 cols remain.
                    nc.scalar.activation(sig[0:100, 24:48], pout[0:100, 24:48],
                                         AF.Sigmoid)
                    nc.sync.dma_start(dout[b * 100 : b * 100 + 100, 24:48],
                                      sig[0:100, 24:48])
                    emit_g3(prev)
                    prev = None
                    nc.scalar.activation(sig[0:100, 48:NCOL], pout[0:100, 48:NCOL],
                                         AF.Sigmoid)
                    nc.sync.dma_start(dout[b * 100 : b * 100 + 100, 48:NCOL],
                                      sig[0:100, 48:NCOL])

    nc.compile()
    return nc


def _host_prep(x, pe_k, pe_v, comp, basis, root, rgcn_bias, W1):
    import ml_dtypes

    bf = ml_dtypes.bfloat16
    f8 = ml_dtypes.float8_e4m3

    ra = _rel_adj(S) % NREL
    onehot = (ra[None, :, :] == np.arange(NREL)[:, None, None]).astype(np.float64)
    deg = onehot.sum(1)
    inv = np.where(deg > 0, 1.0 / np.maximum(deg, 1.0), 0.0)
    anorm = onehot * inv[:, None, :]
    ahat_full = np.einsum("rk,rij->kij", np.asarray(comp, np.float64), anorm)
    ahat_full = ahat_full.astype(np.float32)  # [2, S, S]
    pos = np.clip(np.arange(S)[:, None] - np.arange(S)[None, :] + 1, 0, MAXL)

    x = np.asarray(x, np.float32)
    W1 = np.asarray(W1, np.float32)
    W1a, W1b = W1[:D], W1[D : D + P]
    W1c, W1d = W1[D + P : 2 * D + P], W1[2 * D + P :]
    ttab = (np.asarray(pe_k, np.float64) @ W1b.astype(np.float64)
            + np.asarray(pe_v, np.float64) @ W1d.astype(np.float64)).astype(np.float32)
    ttab_b = ttab.astype(bf).astype(np.float32)  # [11, 512] as the device sees it

    W2 = np.asarray(W2_GLOBAL, np.float32)
    # fp8 half (K rows 0..255): hi + lo residual pair
    W2hi = W2[:256].astype(f8)
    W2lo = (W2[:256] - W2hi.astype(np.float32)).astype(f8)
    w2f8 = np.zeros((128, 2048), f8)
    for s, Wq in enumerate((W2hi, W2lo)):
        for n in range(4):
            for i in range(2):
                w2f8[:, (s * 4 + n) * 256 + i * 128 : (s * 4 + n) * 256 + i * 128 + 128] = \
                    Wq[i * 128 : i * 128 + 128, n * 128 : n * 128 + 128]
    # bf16 half (K rows 256..511)
    w2b = np.zeros((128, 2 * M), np.float32)
    for j in range(2):
        w2b[:, j * M : (j + 1) * M] = W2[(2 + j) * 128 : (3 + j) * 128, :]

    com = {
        "basis": np.concatenate(
            [_pack_k(np.asarray(basis[k], np.float32)) for k in range(2)], axis=1
        ).astype(bf),
        "root": _pack_k(np.asarray(root, np.float32)).astype(bf),
        "w1a": _pack_k(W1a).astype(bf),
        "w1c": _pack_k(W1c).astype(bf),
        "w2f8": w2f8,
        "w2b": w2b.astype(bf),
        "wp": np.ascontiguousarray(np.asarray(WP_GLOBAL, np.float32)[:, 0]
                                   .reshape(4, 128).T).astype(bf),
    }
    bias_p = np.zeros((128, 3), np.float32)
    rb = np.asarray(rgcn_bias, np.float32)
    for c in range(3):
        r = min(128, D - c * 128)
        bias_p[:r, c] = rb[c * 128 : c * 128 + r]
    com["bias"] = bias_p

    SC = S + IPC
    xt_all = x.transpose(2, 0, 1)  # [D, B, S]
    per_core = []
    for c in range(NCORES):
        i0 = c * IPC
        m = dict(com)
        xtc = np.empty((D, B * SC), np.float32)
        for b in range(B):
            xtc[:, b * SC : b * SC + S] = xt_all[:, b, :]
            xtc[:, b * SC + S : (b + 1) * SC] = xt_all[:, b, i0 : i0 + IPC]
        m["xT"] = xtc.astype(bf)
        ah = np.zeros((128, 4 * SC), np.float32)
        for k in range(2):
            for jc in range(2):
                r = 128 if jc == 0 else 72
                base = (k * 2 + jc) * SC
                ah[:r, base : base + S] = ahat_full[k, jc * 128 : jc * 128 + r, :]
                ah[:r, base + S : base + SC] = ahat_full[k, jc * 128 : jc * 128 + r, i0 : i0 + IPC]
        m["ahat"] = ah.astype(bf)
        # peR: [128, mc*FPC + pair] = T[pos(pair), mc*128+p]
        pe_pairs = ttab_b[pos[i0 : i0 + IPC, :].reshape(-1)]  # [FPC, 512]
        m["per"] = np.ascontiguousarray(
            pe_pairs.T.reshape(4, 128, FPC).reshape(512, FPC)
            .reshape(4, 128, FPC).transpose(1, 0, 2).reshape(128, 4 * FPC)).astype(bf)
        per_core.append(m)
    return per_core


W2_GLOBAL = None
WP_GLOBAL = None


def kernel(x, mask, pe_k, pe_v, comp, basis, root, rgcn_bias, W1, W2, Wp,
           _want_results=False, _trace=False):
    global W2_GLOBAL, WP_GLOBAL
    W2_GLOBAL, WP_GLOBAL = W2, Wp

    from concourse.bass_utils import run_bass_kernel_spmd

    if "nc" not in _prog_cache:
        _prog_cache["nc"] = _build_program()
    nc = _prog_cache["nc"]

    in_maps = _host_prep(x, pe_k, pe_v, comp, basis, root, rgcn_bias, W1)
    res = run_bass_kernel_spmd(nc, in_maps, core_ids=list(range(NCORES)),
                               trace=_trace)

    out = np.zeros((B, S, S), np.float32)
    for c in range(NCORES):
        i0 = c * IPC
        arr = np.asarray(res.results[c]["out"], np.float32).reshape(B, 100, NCOL)
        out[:, i0 : i0 + IPC, :] = arr.transpose(0, 2, 1).reshape(B, IPC, S)
    out *= np.asarray(mask, np.float32)
    if _want_results:
        return out, res
    return out

